# revision 29
# baseline (speedup 1.0000x reference)
"""Trainium2 Bass kernel for nn_CSBrainLLMVQ — v3.

Data-parallel over batch: 4 batches/core x 8 cores; no collectives. All
weight-only tensors are folded on the host: the conv/GN weights, the FFT
matrix, CB2T = inp_w^T @ codebook^T (+ nvec norm row) as fp16 hi/lo pairs,
and W2f = codebook @ outp_w^T + outp_b (the per-code output row, gathered
by index from DRAM).

Device pipeline per core:
  1. conv1-3 + GroupNorm + exact GELU (f32r matmuls; GN stats via Scalar
     Square-accumulate + DVE sums; GN-apply+GELU fused into one Scalar
     activation per batch).
  2. FFT magnitude + spectral projection (f32r matmuls).
  3. positional depthwise 19x7 conv as dense 19x19 Toeplitz matmuls over
     the channel axis: dm-chunks of 6 channels go through a DMA relayout
     [(dm h), b, w], 7 shifted matmuls against host-built block-diagonal
     Toeplitz weights (fp16), and a relayout back.
  4. VQ scores: 3-term fp16 hi/lo matmul (exact to ~1e-7) against the
     host-folded CB2T, two-stage argmax (block max8 + in-block find), and
     an indirect-DMA gather of W2f rows.
"""
import numpy as np

B, CH, NP_, PS = 32, 19, 30, 200
DM, LLM, KC = 200, 4096, 4096
EPS = 1e-5
T1 = CH * NP_          # 570 tokens per batch
NB = 4                 # batches per core
TOK = NB * T1          # 2280 tokens per core
NCORES = 8
NSW = 456              # f32r matmul N-slice (5 x 456 = 2280, all >= 256)

# posconv chunking: group A (dm 0..127): 21 chunks of 6 + 1 of 2;
# group B (dm 128..199): 12 chunks of 6. K rows = 19*ndm (<=114), M pad 128.
CHUNKS_A = [(6 * i, 6) for i in range(21)] + [(126, 2)]
CHUNKS_B = [(6 * i, 6) for i in range(12)]
NCHUNK = len(CHUNKS_A) + len(CHUNKS_B)   # 34

_COMPILED = None


def _tok_tiles():
    out, t0 = [], 0
    while t0 < TOK:
        out.append((t0, min(128, TOK - t0)))
        t0 += 128
    return out


def _n_slices(width=NSW):
    out, n0 = [], 0
    while n0 < TOK:
        out.append((n0, min(width, TOK - n0)))
        n0 += width
    return out


def _f16_split(a):
    hi = a.astype(np.float16)
    lo = (a - hi.astype(np.float64)).astype(np.float16)
    return hi, lo


def build_host_weights(inp):
    w = {}
    # conv1 as [201, 200] (row 200 = bias, moved to convb)
    W1 = np.zeros((201, 200), np.float32)
    c1w = np.asarray(inp["c1w"]).reshape(25, 49)
    for c in range(25):
        for o in range(8):
            for t in range(49):
                i = o * 25 - 24 + t
                if 0 <= i < 200:
                    W1[i, c * 8 + o] = c1w[c, t]
    W1[200, :] = np.repeat(np.asarray(inp["c1b"]), 8)
    w["W1big"] = W1

    # conv2/3: NO 0.5 folding (GELU is exact via AF.Gelu now)
    for name, wk, bk in [("W2big", "c2w", "c2b"), ("W3big", "c3w", "c3b")]:
        Wb = np.zeros((201, 200), np.float32)
        cw = np.asarray(inp[wk]).reshape(25, 25, 3)
        for co in range(25):
            for o in range(8):
                for ci in range(25):
                    for t in range(3):
                        oi = o + t - 1
                        if 0 <= oi < 8:
                            Wb[ci * 8 + oi, co * 8 + o] = cw[co, ci, t]
        Wb[200, :] = np.repeat(np.asarray(inp[bk]), 8)
        w[name] = Wb

    k = np.arange(101)[None, :]
    n = np.arange(200)[:, None]
    ang = -2.0 * np.pi * k * n / 200.0
    F = np.zeros((201, 202), np.float64)
    F[:200, :101] = np.cos(ang) / 200.0
    F[:200, 101:] = np.sin(ang) / 200.0
    w["Fcat"] = F.astype(np.float32)

    sw = np.zeros((102, 200), np.float32)
    sw[:101] = np.asarray(inp["spec_w"]).T
    sw[101] = np.asarray(inp["spec_b"])
    w["spec_wT"] = sw

    for i, (sk, bk) in enumerate([("gn1s", "gn1b"), ("gn2s", "gn2b"), ("gn3s", "gn3b")], 1):
        w[f"gn{i}gamma"] = np.repeat(np.asarray(inp[sk]), 8).astype(np.float32).reshape(200, 1)
        w[f"gn{i}beta"] = np.repeat(np.asarray(inp[bk]), 8).astype(np.float32).reshape(200, 1)

    gm = np.zeros((200, 5), np.float32)
    for p in range(200):
        gm[p, p // 40] = 1.0
    w["gmask"] = gm
    w["gmaskT"] = np.ascontiguousarray(gm.T)

    w["posb"] = np.asarray(inp["pos_b"]).astype(np.float32).reshape(200, 1)
    w["convb"] = np.stack([w["W1big"][200], w["W2big"][200], w["W3big"][200]],
                          1).astype(np.float32)

    # packed FE weights: FEW = [W1 | W2 | W3 | Fcat] rows split 0:128 / 128:200
    few = np.concatenate([w["W1big"][:, :], w["W2big"][:, :], w["W3big"][:, :],
                          w["Fcat"]], 1)       # [201, 802]
    w["FEW_A"] = np.ascontiguousarray(few[0:128]).astype(np.float32)
    w["FEW_B"] = np.ascontiguousarray(few[128:200]).astype(np.float32)
    # packed per-partition params: [convb(3) | gn g/b x3 (6) | gmask(5) | posb(1)]
    prm = np.concatenate(
        [w["convb"],
         w["gn1gamma"], w["gn1beta"], w["gn2gamma"], w["gn2beta"],
         w["gn3gamma"], w["gn3beta"], w["gmask"], w["posb"]], 1)  # [200, 15]
    w["PRM_A"] = np.ascontiguousarray(prm[0:128]).astype(np.float32)
    w["PRM_B"] = np.ascontiguousarray(prm[128:200]).astype(np.float32)

    # posconv Toeplitz blocks: per chunk 14 mats ([114,128] fp16 block-diag):
    # 0..6 = Hh per dx, 7..13 = Hl per dx; lhsT[(d,h'),(d,h)] = W[dm0+d, h'-h+9, dx]
    posw = np.asarray(inp["pos_w"]).reshape(200, 19, 7).astype(np.float64)
    pwh = posw.astype(np.float16).astype(np.float64)
    pwl = posw - pwh
    hp_, h_ = np.meshgrid(np.arange(19), np.arange(19), indexing="ij")
    dy_ = hp_ - h_ + 9
    valid = (dy_ >= 0) & (dy_ < 19)
    dyc = np.clip(dy_, 0, 18)
    hst = np.zeros((NCHUNK * 14, 114, 128), np.float16)
    ci = 0
    for base, chunks in [(0, CHUNKS_A), (128, CHUNKS_B)]:
        for (off, ndm) in chunks:
            dm0 = base + off
            for hi_lo, W in [(0, pwh), (7, pwl)]:
                for dx in range(7):
                    M = np.zeros((114, 128), np.float64)
                    for d in range(ndm):
                        blk = np.where(valid, W[dm0 + d][dyc, dx], 0.0)
                        M[d * 19:(d + 1) * 19, d * 19:(d + 1) * 19] = blk
                    hst[ci * 14 + hi_lo + dx] = M.astype(np.float16)
            ci += 1
    w["Hst"] = hst.reshape(NCHUNK * 14 * 114, 128)

    # CB2T + nvec (fp64 host fold), fp16 hi/lo splits
    iw = np.asarray(inp["inp_w"]).astype(np.float64)
    cb = np.asarray(inp["codebook"]).astype(np.float64)
    cb2 = iw.T @ cb.T                                     # [200, 4096]
    nvec = cb @ np.asarray(inp["inp_b"]).astype(np.float64) - 0.5 * (cb * cb).sum(-1)
    cb2h, cb2l = _f16_split(cb2)
    nvh, nvl = _f16_split(nvec)
    w["cbhA"] = np.ascontiguousarray(cb2h[0:128])
    w["cblA"] = np.ascontiguousarray(cb2l[0:128])
    cbhB = np.zeros((97, KC), np.float16)
    cblB = np.zeros((97, KC), np.float16)
    cbhB[0:72] = cb2h[128:200]
    cbhB[96] = nvh
    cblB[0:72] = cb2l[128:200]
    cblB[96] = nvl
    w["cbhB"] = cbhB
    w["cblB"] = cblB

    # W2f rows (gathered by code index), outp_b folded in
    ow = np.asarray(inp["outp_w"]).astype(np.float64)
    w["W2f"] = (cb @ ow.T + np.asarray(inp["outp_b"]).astype(np.float64)).astype(np.float32)

    w["iota512"] = np.tile(np.arange(512, dtype=np.uint16), (128, 1))
    return w


def _build_nc(debug=False):
    import concourse.bass as bass
    import concourse.mybir as mybir
    import concourse.tile as tile
    from concourse import bacc

    f32 = mybir.dt.float32
    f32r = mybir.dt.float32r
    f16 = mybir.dt.float16
    u16 = mybir.dt.uint16
    u32 = mybir.dt.uint32
    Alu = mybir.AluOpType
    AF = mybir.ActivationFunctionType
    AX = mybir.AxisListType.X

    nc = bacc.Bacc("TRN2", target_bir_lowering=False, debug=False, num_devices=NCORES)

    di = {}
    di["xT"] = nc.dram_tensor("xT", [200, TOK], f32, kind="ExternalInput")
    di["FEW_A"] = nc.dram_tensor("FEW_A", [128, 802], f32, kind="ExternalInput")
    di["FEW_B"] = nc.dram_tensor("FEW_B", [72, 802], f32, kind="ExternalInput")
    di["PRM_A"] = nc.dram_tensor("PRM_A", [128, 15], f32, kind="ExternalInput")
    di["PRM_B"] = nc.dram_tensor("PRM_B", [72, 15], f32, kind="ExternalInput")
    di["spec_wT"] = nc.dram_tensor("spec_wT", [102, 200], f32, kind="ExternalInput")
    di["gmaskT"] = nc.dram_tensor("gmaskT", [5, 200], f32, kind="ExternalInput")
    di["Hst"] = nc.dram_tensor("Hst", [NCHUNK * 14 * 114, 128], f16, kind="ExternalInput")
    di["cbhA"] = nc.dram_tensor("cbhA", [128, KC], f16, kind="ExternalInput")
    di["cblA"] = nc.dram_tensor("cblA", [128, KC], f16, kind="ExternalInput")
    di["cbhB"] = nc.dram_tensor("cbhB", [97, KC], f16, kind="ExternalInput")
    di["cblB"] = nc.dram_tensor("cblB", [97, KC], f16, kind="ExternalInput")
    di["W2f"] = nc.dram_tensor("W2f", [KC, 200], f32, kind="ExternalInput")
    di["iota512"] = nc.dram_tensor("iota512", [128, 512], u16, kind="ExternalInput")
    p16d = nc.dram_tensor("p16d", [DM * 19, 2 * NB * 30], f16, kind="Internal")
    yd = nc.dram_tensor("yd", [DM * 19, NB * 30], f32, kind="Internal")
    # layouts: p16d rows (dm*19+h), cols (hl, w, b); yd rows same, cols (w, b)

    out_d = nc.dram_tensor("out", [TOK, 200], f32, kind="ExternalOutput")
    idx_d = nc.dram_tensor("idx", [128, 18], u32, kind="ExternalOutput")
    dbg = {}
    if debug:
        for nm in ["d_pe1", "d_pe2", "d_g1"]:
            dbg[nm] = nc.dram_tensor(nm, [200, TOK], f32, kind="ExternalOutput")
        dbg["d_sc0"] = nc.dram_tensor("d_sc0", [128, KC], f32, kind="ExternalOutput")
        dbg["d_am0"] = nc.dram_tensor("d_am0", [128, 40], f32, kind="ExternalOutput")

    TT = _tok_tiles()
    NS = _n_slices()

    with tile.TileContext(nc) as tc:
        with (
            tc.tile_pool(name="persist", bufs=1)) as persist, (
            tc.tile_pool(name="cbpool", bufs=1)) as cbpool, (
            tc.tile_pool(name="pepool", bufs=1)) as pepool:
            gidxu = persist.tile([128, 18], u32, name="gidxu")

            # score tables (loaded via the idle gpsimd queue; needed late)
            cbhA = cbpool.tile([128, KC], f16, name="cbhA")
            cblA = cbpool.tile([128, KC], f16, name="cblA")
            cbhB = cbpool.tile([97, KC], f16, name="cbhB")
            cblB = cbpool.tile([97, KC], f16, name="cblB")
            for nm, t in [("cbhA", cbhA), ("cblA", cblA), ("cbhB", cbhB), ("cblB", cblB)]:
                nc.gpsimd.dma_start(t[:], di[nm][:])

            # pe'' fp16 splits (score matmul lhs)
            pehA = pepool.tile([128, TOK], f16, name="pehA")
            pelA = pepool.tile([128, TOK], f16, name="pelA")
            pehB = pepool.tile([97, TOK], f16, name="pehB")
            pelB = pepool.tile([97, TOK], f16, name="pelB")
            nc.vector.memset(pehB[64:96, :], 0.0)
            nc.vector.memset(pehB[96:97, :], 1.0)
            nc.vector.memset(pelB[64:96, :], 0.0)
            nc.vector.memset(pelB[96:97, :], 0.0)

            # ---------------- Front end ----------------
            with (
                tc.tile_pool(name="fe2", bufs=1) as fe2,
                tc.tile_pool(name="fetmp", bufs=2) as fetmp,
                tc.tile_pool(name="fe1", bufs=1) as fe1,
                tc.tile_pool(name="feps", bufs=3, space="PSUM") as feps,
                tc.tile_pool(name="stps", bufs=1, space="PSUM") as stps,
            ):
                xA = fe1.tile([128, TOK], f32, name="xA", tag="xA")
                xB = fe1.tile([72, TOK], f32, name="xB", tag="xB")
                fewA = fe2.tile([128, 802], f32, name="fewA", tag="fewA")
                fewB = fe2.tile([72, 802], f32, name="fewB", tag="fewB")
                prmA = fetmp.tile([128, 15], f32, name="prmA", tag="prmA")
                prmB = fetmp.tile([72, 15], f32, name="prmB", tag="prmB")
                gmT = fetmp.tile([5, 200], f32, name="gmT", tag="gmT")
                nc.sync.dma_start(xA[:, 0:NSW], di["xT"][0:128, 0:NSW])
                nc.sync.dma_start(xB[:, 0:NSW], di["xT"][128:200, 0:NSW])
                nc.sync.dma_start(fewA[:], di["FEW_A"][:])
                nc.sync.dma_start(fewB[:], di["FEW_B"][:])
                for (n0, nsz) in NS[1:]:
                    nc.sync.dma_start(xA[:, n0:n0 + nsz], di["xT"][0:128, n0:n0 + nsz])
                    nc.sync.dma_start(xB[:, n0:n0 + nsz], di["xT"][128:200, n0:n0 + nsz])
                nc.scalar.dma_start(prmA[:], di["PRM_A"][:])
                nc.scalar.dma_start(prmB[:], di["PRM_B"][:])
                nc.scalar.dma_start(gmT[:], di["gmaskT"][:])
                gmA = prmA[:, 9:14]
                gmB = prmB[:, 9:14]

                g1A = fe2.tile([128, TOK], f32, name="g1A", tag="gA1")
                g1B = fe2.tile([72, TOK], f32, name="g1B", tag="gB1")
                g2A = fe2.tile([128, TOK], f32, name="g2A", tag="gA2")
                g2B = fe2.tile([72, TOK], f32, name="g2B", tag="gB2")
                g3A = fe2.tile([128, TOK], f32, name="g3A", tag="gA1")
                g3B = fe2.tile([72, TOK], f32, name="g3B", tag="gB1")

                def conv_gn_gelu(rhsA, rhsB, wcol, gi, outA, outB, dbg_g=None):
                    """rhs [128/72, TOK] f32 -> out = gelu(GN(conv)) f32."""
                    WA = fewA[:, wcol:wcol + 200]
                    WB = fewB[:, wcol:wcol + 200]
                    bcA = prmA[:, gi - 1:gi]
                    bcB = prmB[:, gi - 1:gi]
                    gamA = prmA[:, 1 + 2 * gi:2 + 2 * gi]
                    gamB = prmB[:, 1 + 2 * gi:2 + 2 * gi]
                    betA = prmA[:, 2 + 2 * gi:3 + 2 * gi]
                    betB = prmB[:, 2 + 2 * gi:3 + 2 * gi]

                    convA = fe1.tile([128, TOK], f32, name=f"convA{gi}", tag="convA")
                    convB = fe1.tile([72, TOK], f32, name=f"convB{gi}", tag="convB")
                    for (m0, msz, cdst, bc) in [(0, 128, convA, bcA),
                                                (128, 72, convB, bcB)]:
                        for (n0, nsz) in NS:
                            cps = feps.tile([128, NSW], f32, name="cps", tag="cps")
                            nc.tensor.matmul(cps[:msz, :nsz], WA[:, m0:m0 + msz],
                                             rhsA[:, n0:n0 + nsz], start=True, stop=False)
                            nc.tensor.matmul(cps[:msz, :nsz], WB[:, m0:m0 + msz],
                                             rhsB[:, n0:n0 + nsz], start=False, stop=True)
                            nc.scalar.activation(cdst[:, n0:n0 + nsz], cps[:msz, :nsz],
                                                 AF.Identity, bias=bc[:msz, :])

                    # GN stats: sum via DVE reduce, sumsq via Scalar Square-accum
                    stA = fetmp.tile([128, 8], f32, name=f"stA{gi}", tag="stA")
                    stB = fetmp.tile([72, 8], f32, name=f"stB{gi}", tag="stB")
                    scrA = fe1.tile([128, T1], f32, name=f"scrA{gi}", tag="scrA")
                    scrB = fe1.tile([72, T1], f32, name=f"scrB{gi}", tag="scrB")
                    for b in range(NB):
                        sl = slice(b * T1, (b + 1) * T1)
                        nc.vector.reduce_sum(stA[:, 2 * b:2 * b + 1], convA[:, sl], axis=AX)
                        nc.vector.reduce_sum(stB[:, 2 * b:2 * b + 1], convB[:, sl], axis=AX)
                        nc.scalar.activation(scrA[:], convA[:, sl], AF.Square,
                                             accum_out=stA[:, 2 * b + 1:2 * b + 2])
                        nc.scalar.activation(scrB[:], convB[:, sl], AF.Square,
                                             accum_out=stB[:, 2 * b + 1:2 * b + 2])
                    sps = stps.tile([5, 8], f32, name="sps", tag="stp")
                    nc.tensor.matmul(sps[:], gmA[:], stA[:], start=True, stop=False)
                    nc.tensor.matmul(sps[:], gmB[:], stB[:], start=False, stop=True)

                    st = fetmp.tile([5, 16], f32, name=f"st{gi}", tag="st")
                    st2 = fetmp.tile([5, 8], f32, name=f"st2{gi}", tag="st2")
                    NINV = 1.0 / (40 * T1)
                    nc.vector.tensor_scalar(st[:, 0:8], sps[:], NINV, None, op0=Alu.mult)
                    for b in range(NB):
                        nc.vector.tensor_copy(st2[:, b:b + 1], st[:, 2 * b:2 * b + 1])
                        nc.vector.tensor_mul(st[:, 8 + b:9 + b], st[:, 2 * b:2 * b + 1],
                                             st[:, 2 * b:2 * b + 1])
                        nc.vector.tensor_sub(st2[:, 4 + b:5 + b], st[:, 2 * b + 1:2 * b + 2],
                                             st[:, 8 + b:9 + b])
                    nc.vector.tensor_scalar(st2[:, 4:8], st2[:, 4:8], EPS, None, op0=Alu.add)
                    sqr = fetmp.tile([5, 4], f32, name=f"sqr{gi}", tag="sqr")
                    nc.scalar.activation(sqr[:], st2[:, 4:8], AF.Sqrt)
                    r0 = fetmp.tile([5, 4], f32, name=f"r0{gi}", tag="r0")
                    nc.vector.reciprocal(r0[:], sqr[:])
                    tn = fetmp.tile([5, 4], f32, name=f"tn{gi}", tag="tn")
                    nc.vector.tensor_mul(tn[:], r0[:], r0[:])
                    nc.vector.tensor_mul(tn[:], tn[:], st2[:, 4:8])
                    nc.vector.tensor_scalar(tn[:], tn[:], -0.5, 1.5, op0=Alu.mult, op1=Alu.add)
                    nc.vector.tensor_mul(st2[:, 4:8], r0[:], tn[:])

                    bpsA = stps.tile([128, 8], f32, name="bpsA", tag="stp")
                    bpsB = stps.tile([72, 8], f32, name="bpsB", tag="stp")
                    nc.tensor.matmul(bpsA[:], gmT[:, 0:128], st2[:], start=True, stop=True)
                    nc.tensor.matmul(bpsB[:], gmT[:, 128:200], st2[:], start=True, stop=True)
                    rgA = fetmp.tile([128, 8], f32, name=f"rgA{gi}", tag="rgA")
                    rgB = fetmp.tile([72, 8], f32, name=f"rgB{gi}", tag="rgB")
                    for (bps, rg, gmv, btv, prt) in [(bpsA, rgA, gamA, betA, 128),
                                                     (bpsB, rgB, gamB, betB, 72)]:
                        # rg[0:4] = rstd*gamma; rg[4:8] = beta - mean*rstd*gamma
                        nc.vector.tensor_scalar(rg[:prt, 0:4], bps[:prt, 4:8],
                                                gmv[:prt, :], None, op0=Alu.mult)
                        nc.vector.tensor_mul(rg[:prt, 4:8], bps[:prt, 0:4], rg[:prt, 0:4])
                        nc.vector.tensor_scalar(rg[:prt, 4:8], rg[:prt, 4:8],
                                                btv[:prt, :], None, op0=Alu.subtract)
                        nc.vector.tensor_scalar(rg[:prt, 4:8], rg[:prt, 4:8], -1.0, None,
                                                op0=Alu.mult)
                    # fused GN-apply + exact GELU on Scalar engine
                    for b in range(NB):
                        sl = slice(b * T1, (b + 1) * T1)
                        nc.scalar.activation(outA[:, sl], convA[:, sl], AF.Gelu,
                                             scale=rgA[:, b:b + 1], bias=rgA[:, 4 + b:5 + b])
                        nc.scalar.activation(outB[:, sl], convB[:, sl], AF.Gelu,
                                             scale=rgB[:, b:b + 1], bias=rgB[:, 4 + b:5 + b])
                    if dbg_g is not None:
                        nc.sync.dma_start(dbg_g[0:128, :], outA[:])
                        nc.sync.dma_start(dbg_g[128:200, :], outB[:])

                FA = fewA[:, 600:802]
                FB = fewB[:, 600:802]
                reT = fe2.tile([101, TOK], f32, name="reT", tag="gA2p",
                               padded_shape=[128, TOK])
                imT = fe2.tile([101, TOK], f32, name="imT", tag="gB2p",
                               padded_shape=[128, TOK])

                conv_gn_gelu(xA, xB, 0, 1, g1A, g1B, dbg.get("d_g1"))
                for (m0, dst) in [(0, reT), (101, imT)]:
                    for (n0, nsz) in NS:
                        cps = feps.tile([128, NSW], f32, name="cpsf", tag="cps")
                        nc.tensor.matmul(cps[:101, :nsz], FA[:, m0:m0 + 101],
                                         xA[:, n0:n0 + nsz], start=True, stop=False)
                        nc.tensor.matmul(cps[:101, :nsz], FB[:, m0:m0 + 101],
                                         xB[:, n0:n0 + nsz], start=False, stop=True)
                        nc.scalar.activation(dst[:, n0:n0 + nsz], cps[:101, :nsz], AF.Copy)
                conv_gn_gelu(g1A, g1B, 200, 2, g2A, g2B)
                nc.vector.tensor_mul(reT[:], reT[:], reT[:])
                nc.vector.tensor_mul(imT[:], imT[:], imT[:])
                nc.vector.tensor_add(reT[:], reT[:], imT[:])
                conv_gn_gelu(g2A, g2B, 400, 3, g3A, g3B)

                specA = fe1.tile([102, TOK], f32, name="specA", tag="convA")
                nc.vector.memset(specA[96:102, :], 1.0)
                epsb = fetmp.tile([101, 1], f32, name="epsb", tag="gam")
                nc.vector.memset(epsb[:], 1e-30)
                nc.scalar.activation(specA[0:101, :], reT[:], AF.Sqrt, bias=epsb[:])
                swT = fetmp.tile([102, 200], f32, name="swT", tag="WB")
                nc.scalar.dma_start(swT[:], di["spec_wT"][0:102, :])
                pe1A = fe2.tile([128, TOK], f32, name="pe1A", tag="gA2p",
                                padded_shape=[128, TOK])
                pe1B = fe2.tile([72, TOK], f32, name="pe1B", tag="gB2p",
                                padded_shape=[128, TOK])
                for (m0, msz, gsrc, pdst) in [(0, 128, g3A, pe1A), (128, 72, g3B, pe1B)]:
                    for (n0, nsz) in NS:
                        cps = feps.tile([128, NSW], f32, name="cpss", tag="cps")
                        nc.tensor.matmul(cps[:msz, :nsz], swT[:, m0:m0 + msz],
                                         specA[:, n0:n0 + nsz], start=True, stop=True)
                        nc.vector.scalar_tensor_tensor(
                            pdst[:, n0:n0 + nsz], cps[:msz, :nsz], 1.0,
                            gsrc[:msz, n0:n0 + nsz],
                            op0=Alu.mult, op1=Alu.add)
                if debug:
                    nc.sync.dma_start(dbg["d_pe1"][0:128, :], pe1A[:])
                    nc.sync.dma_start(dbg["d_pe1"][128:200, :], pe1B[:])

                # ---------------- pos conv (Toeplitz h-matmuls) ----------------
                pbA = prmA[:, 14:15]
                pbB = prmB[:, 14:15]
                # fp16 hi/lo of pe1 stored [dm, h, w, b] (batch innermost) so the
                # DRAM staging runs are (w, b) = 240B contiguous
                pe16A = fe1.tile([128, 19, 30, NB], f16, name="pe16A", tag="scrA2",
                                 padded_shape=[128, 19, 30, NB])
                pe16B = fe1.tile([72, 19, 30, NB], f16, name="pe16B", tag="scrB2",
                                 padded_shape=[128, 19, 30, NB])
                pl16A = fe1.tile([128, 19, 30, NB], f16, name="pl16A", tag="scrA3",
                                 padded_shape=[128, 19, 30, NB])
                pl16B = fe1.tile([72, 19, 30, NB], f16, name="pl16B", tag="scrB3",
                                 padded_shape=[128, 19, 30, NB])
                for (p16, pl16, pe1x, nb) in [(pe16A, pl16A, pe1A, 128),
                                              (pe16B, pl16B, pe1B, 72)]:
                    hv16 = p16[:].rearrange("d h w b -> d b (h w)")
                    lv16 = pl16[:].rearrange("d h w b -> d b (h w)")
                    pv = pe1x[:].rearrange("d (b hw) -> d b hw", b=NB)
                    nc.scalar.activation(hv16, pv, AF.Copy)
                    nc.vector.tensor_tensor(lv16, pv, hv16, op=Alu.subtract)
                posPA = fe2.tile([128, 19, 30, NB], f32, name="posPA", tag="gA1")
                posPB = fe2.tile([72, 19, 30, NB], f32, name="posPB", tag="gB1")

                # stage Xh/Xl to DRAM: [(dm h), (hl, w, b)]; 4 DMAs, 240B runs
                p16v = p16d[:].rearrange("(d h) (l n) -> d h l n", h=19, l=2)
                ydv = yd[:].rearrange("(d h) n -> d h n", h=19)
                for (hl, srcA, srcB) in [(0, pe16A, pe16B), (1, pl16A, pl16B)]:
                    nc.sync.dma_start(
                        p16v[0:128, :, hl, :],
                        srcA[:].rearrange("d h w b -> d h (w b)"))
                    nc.scalar.dma_start(
                        p16v[128:200, :, hl, :],
                        srcB[:].rearrange("d h w b -> d h (w b)"))
                with (
                    tc.tile_pool(name="pcx", bufs=7) as pcx,
                    tc.tile_pool(name="pch", bufs=6) as pch,
                    tc.tile_pool(name="pcy", bufs=4) as pcy,
                    tc.tile_pool(name="pcps", bufs=4, space="PSUM") as pcps,
                ):
                    hview = di["Hst"][:].rearrange("(c p) m -> c p m", p=114)
                    p16r = p16d[:].rearrange("r (l n) -> r l n", l=2)
                    ci = 0
                    for (base, chunks) in [(0, CHUNKS_A), (128, CHUNKS_B)]:
                        for (off, ndm) in chunks:
                            rows = ndm * 19
                            r0 = (base + off) * 19
                            # Xc layout [114, hl, w(36 padded), b]
                            Xc = pcx.tile([114, 2, 36, NB], f16, name="Xc", tag="Xc")
                            nc.vector.memset(Xc[:, :, 0:3, :], 0.0)
                            nc.vector.memset(Xc[:, :, 33:36, :], 0.0)
                            if rows < 114:
                                nc.vector.memset(Xc[32:64, :, :, :], 0.0)
                                nc.vector.memset(Xc[64:96, :, :, :], 0.0)
                                nc.vector.memset(Xc[96:114, :, :, :], 0.0)
                            nc.sync.dma_start(
                                Xc[0:rows, :, 3:33, :].rearrange("p l w b -> p l (w b)"),
                                p16r[r0:r0 + rows, :, :])
                            Hc = pch.tile([114, 14, 128], f16, name="Hc", tag="Hc")
                            nc.scalar.dma_start(
                                Hc[:], hview[14 * ci:14 * ci + 14, :, :]
                                .rearrange("c p m -> p c m"))
                            pc = pcps.tile([128, 30, NB], f32, name="pc", tag="pc")
                            # Hh*Xh + Hh*Xl (same weights back-to-back), + Hl*Xh
                            for dx in range(7):
                                nc.tensor.matmul(pc[:], Hc[:, dx, :],
                                                 Xc[:, 0, dx:dx + 30, :],
                                                 start=(dx == 0), stop=False)
                                nc.tensor.matmul(pc[:], Hc[:, dx, :],
                                                 Xc[:, 1, dx:dx + 30, :],
                                                 start=False, stop=False)
                            for dx in range(7):
                                nc.tensor.matmul(pc[:], Hc[:, 7 + dx, :],
                                                 Xc[:, 0, dx:dx + 30, :],
                                                 start=False, stop=(dx == 6))
                            Yc = pcy.tile([114, 30, NB], f32, name="Yc", tag="Yc")
                            nc.scalar.activation(Yc[:], pc[0:114, :, :], AF.Copy)
                            nc.gpsimd.dma_start(
                                yd[r0:r0 + rows, :],
                                Yc[:rows].rearrange("p w b -> p (w b)"))
                            ci += 1
                    nc.sync.dma_start(
                        posPA[:].rearrange("d h w b -> d h (w b)"), ydv[0:128, :, :])
                    nc.sync.dma_start(
                        posPB[:].rearrange("d h w b -> d h (w b)"), ydv[128:200, :, :])

                # pe'' = pe1 + pos + posb; then fp16 hi/lo split
                pe2A = fe2.tile([128, TOK], f32, name="pe2A", tag="gA2")
                pe2B = fe2.tile([72, TOK], f32, name="pe2B", tag="gB2")
                nc.vector.scalar_tensor_tensor(
                    pe2A[:].rearrange("d (b hw) -> d b hw", b=NB),
                    posPA[:].rearrange("d h w b -> d b (h w)"), pbA[:, 0:1],
                    pe1A[:].rearrange("d (b hw) -> d b hw", b=NB),
                    op0=Alu.add, op1=Alu.add)
                nc.vector.scalar_tensor_tensor(
                    pe2B[:].rearrange("d (b hw) -> d b hw", b=NB),
                    posPB[:].rearrange("d h w b -> d b (h w)"), pbB[:, 0:1],
                    pe1B[:].rearrange("d (b hw) -> d b hw", b=NB),
                    op0=Alu.add, op1=Alu.add)
                if debug:
                    nc.sync.dma_start(dbg["d_pe2"][0:128, :], pe2A[:])
                    nc.sync.dma_start(dbg["d_pe2"][128:200, :], pe2B[:])
                nc.scalar.activation(pehA[:], pe2A[:], AF.Copy)
                nc.vector.tensor_sub(pelA[:], pe2A[:], pehA[:])
                nc.scalar.activation(pehB[0:72, :], pe2B[:], AF.Copy)
                nc.vector.tensor_sub(pelB[0:72, :], pe2B[:], pehB[0:72, :])

            # ------- scores: 3-term fp16, 2-stage argmax, W2f gather
            with (
                tc.tile_pool(name="sce", bufs=2) as sce,
                tc.tile_pool(name="gat", bufs=3) as gat,
                tc.tile_pool(name="scps", bufs=8, space="PSUM") as scps,
            ):
                for ti, (t0, tsz) in enumerate(TT):
                    tsl = slice(t0, t0 + tsz)
                    sc = sce.tile([128, KC], f32, name="sc", tag="sc")
                    for kc in range(8):
                        csl = slice(kc * 512, (kc + 1) * 512)
                        sps_ = scps.tile([128, 512], f32, name="sps_", tag="sps")
                        seq = [
                            (pehA, cbhA), (pelA, cbhA), (pehA, cblA),
                            (pehB, cbhB), (pelB, cbhB), (pehB, cblB),
                        ]
                        for i, (lh, rh) in enumerate(seq):
                            nc.tensor.matmul(sps_[:tsz, :], lh[:, tsl], rh[:, csl],
                                             start=(i == 0), stop=(i == len(seq) - 1))
                        nc.scalar.activation(sc[:tsz, csl], sps_[:tsz, :], AF.Copy)
                    # argmax: top-8 values + index find (hidden under PE)
                    m8 = gat.tile([128, 8], f32, name="m8", tag="m8")
                    mi8 = gat.tile([128, 8], u32, name="mi8", tag="mi8")
                    nc.vector.max(m8[:tsz, :], sc[:tsz, :])
                    nc.vector.max_index(mi8[:tsz, :], m8[:tsz, :], sc[:tsz, :])
                    nc.vector.tensor_copy(gidxu[:tsz, ti:ti + 1], mi8[:tsz, 0:1])
                    if debug and ti == 0:
                        nc.sync.dma_start(dbg["d_sc0"][:], sc[:])
                    go = gat.tile([128, 200], f32, name="go", tag="go")
                    nc.gpsimd.indirect_dma_start(
                        out=go[:tsz, :], out_offset=None,
                        in_=di["W2f"][:],
                        in_offset=bass.IndirectOffsetOnAxis(
                            ap=gidxu[:tsz, ti:ti + 1], axis=0))
                    nc.sync.dma_start(out_d[t0:t0 + tsz, :], go[:tsz, :])
                nc.sync.dma_start(idx_d[:], gidxu[:])

    nc.compile()
    return nc


def _prep_inputs(inp):
    w = build_host_weights(inp)
    x = np.asarray(inp["x"], np.float32).reshape(B * T1, 200)
    shared = {}
    for k in ["FEW_A", "FEW_B", "PRM_A", "PRM_B", "spec_wT", "gmaskT",
              "Hst", "cbhA", "cblA", "cbhB", "cblB", "W2f", "iota512"]:
        shared[k] = np.ascontiguousarray(w[k])
    in_maps = []
    for c in range(NCORES):
        m = dict(shared)
        m["xT"] = np.ascontiguousarray(x[c * TOK:(c + 1) * TOK].T)
        in_maps.append(m)
    return in_maps


def run(inp, debug=False, trace=False, **kw):
    global _COMPILED
    from concourse.bass_utils import run_bass_kernel_spmd
    if _COMPILED is None or _COMPILED[1] != debug:
        _COMPILED = (_build_nc(debug=debug), debug)
    nc = _COMPILED[0]
    in_maps = _prep_inputs(inp)
    res = run_bass_kernel_spmd(nc, in_maps, core_ids=list(range(NCORES)), trace=trace, **kw)
    return res


def kernel(**inputs):
    res = run(inputs)
    out = np.concatenate([r["out"] for r in res.results], 0)
    return out.reshape(B, CH, NP_, DM)


# revision 30
# speedup vs baseline: 1.0479x; 1.0479x over previous
"""Trainium2 Bass kernel for nn_CSBrainLLMVQ — v3.

Data-parallel over batch: 4 batches/core x 8 cores; no collectives. All
weight-only tensors are folded on the host: the conv/GN weights, the FFT
matrix, CB2T = inp_w^T @ codebook^T (+ nvec norm row) as fp16 hi/lo pairs,
and W2f = codebook @ outp_w^T + outp_b (the per-code output row, gathered
by index from DRAM).

Device pipeline per core:
  1. conv1-3 + GroupNorm + exact GELU (f32r matmuls; GN stats via Scalar
     Square-accumulate + DVE sums; GN-apply+GELU fused into one Scalar
     activation per batch).
  2. FFT magnitude + spectral projection (f32r matmuls).
  3. positional depthwise 19x7 conv as dense 19x19 Toeplitz matmuls over
     the channel axis: dm-chunks of 6 channels go through a DMA relayout
     [(dm h), b, w], 7 shifted matmuls against host-built block-diagonal
     Toeplitz weights (fp16), and a relayout back.
  4. VQ scores: 3-term fp16 hi/lo matmul (exact to ~1e-7) against the
     host-folded CB2T, two-stage argmax (block max8 + in-block find), and
     an indirect-DMA gather of W2f rows.
"""
import numpy as np

B, CH, NP_, PS = 32, 19, 30, 200
DM, LLM, KC = 200, 4096, 4096
EPS = 1e-5
T1 = CH * NP_          # 570 tokens per batch
NB = 4                 # batches per core
TOK = NB * T1          # 2280 tokens per core
NCORES = 8
NSW = 456              # f32r matmul N-slice (5 x 456 = 2280, all >= 256)

# posconv chunking: group A (dm 0..127): 21 chunks of 6 + 1 of 2;
# group B (dm 128..199): 12 chunks of 6. K rows = 19*ndm (<=114), M pad 128.
CHUNKS_A = [(6 * i, 6) for i in range(21)] + [(126, 2)]
CHUNKS_B = [(6 * i, 6) for i in range(12)]
NCHUNK = len(CHUNKS_A) + len(CHUNKS_B)   # 34

_COMPILED = None


def _tok_tiles():
    out, t0 = [], 0
    while t0 < TOK:
        out.append((t0, min(128, TOK - t0)))
        t0 += 128
    return out


def _n_slices(width=NSW):
    out, n0 = [], 0
    while n0 < TOK:
        out.append((n0, min(width, TOK - n0)))
        n0 += width
    return out


def _f16_split(a):
    hi = a.astype(np.float16)
    lo = (a - hi.astype(np.float64)).astype(np.float16)
    return hi, lo


def build_host_weights(inp):
    w = {}
    # conv1 as [201, 200] (row 200 = bias, moved to convb)
    W1 = np.zeros((201, 200), np.float32)
    c1w = np.asarray(inp["c1w"]).reshape(25, 49)
    for c in range(25):
        for o in range(8):
            for t in range(49):
                i = o * 25 - 24 + t
                if 0 <= i < 200:
                    W1[i, c * 8 + o] = c1w[c, t]
    W1[200, :] = np.repeat(np.asarray(inp["c1b"]), 8)
    w["W1big"] = W1

    # conv2/3: NO 0.5 folding (GELU is exact via AF.Gelu now)
    for name, wk, bk in [("W2big", "c2w", "c2b"), ("W3big", "c3w", "c3b")]:
        Wb = np.zeros((201, 200), np.float32)
        cw = np.asarray(inp[wk]).reshape(25, 25, 3)
        for co in range(25):
            for o in range(8):
                for ci in range(25):
                    for t in range(3):
                        oi = o + t - 1
                        if 0 <= oi < 8:
                            Wb[ci * 8 + oi, co * 8 + o] = cw[co, ci, t]
        Wb[200, :] = np.repeat(np.asarray(inp[bk]), 8)
        w[name] = Wb

    k = np.arange(101)[None, :]
    n = np.arange(200)[:, None]
    ang = -2.0 * np.pi * k * n / 200.0
    F = np.zeros((201, 202), np.float64)
    F[:200, :101] = np.cos(ang) / 200.0
    F[:200, 101:] = np.sin(ang) / 200.0
    w["Fcat"] = F.astype(np.float32)

    sw = np.zeros((102, 200), np.float32)
    sw[:101] = np.asarray(inp["spec_w"]).T
    sw[101] = np.asarray(inp["spec_b"])
    w["spec_wT"] = sw

    for i, (sk, bk) in enumerate([("gn1s", "gn1b"), ("gn2s", "gn2b"), ("gn3s", "gn3b")], 1):
        w[f"gn{i}gamma"] = np.repeat(np.asarray(inp[sk]), 8).astype(np.float32).reshape(200, 1)
        w[f"gn{i}beta"] = np.repeat(np.asarray(inp[bk]), 8).astype(np.float32).reshape(200, 1)

    gm = np.zeros((200, 5), np.float32)
    for p in range(200):
        gm[p, p // 40] = 1.0
    w["gmask"] = gm
    w["gmaskT"] = np.ascontiguousarray(gm.T)

    w["posb"] = np.asarray(inp["pos_b"]).astype(np.float32).reshape(200, 1)
    w["convb"] = np.stack([w["W1big"][200], w["W2big"][200], w["W3big"][200]],
                          1).astype(np.float32)

    # packed FE weights: FEW = [W1 | W2 | W3 | Fcat] rows split 0:128 / 128:200
    few = np.concatenate([w["W1big"][:, :], w["W2big"][:, :], w["W3big"][:, :],
                          w["Fcat"]], 1)       # [201, 802]
    w["FEW_A"] = np.ascontiguousarray(few[0:128]).astype(np.float32)
    w["FEW_B"] = np.ascontiguousarray(few[128:200]).astype(np.float32)
    # packed per-partition params: [convb(3) | gn g/b x3 (6) | gmask(5) | posb(1)]
    prm = np.concatenate(
        [w["convb"],
         w["gn1gamma"], w["gn1beta"], w["gn2gamma"], w["gn2beta"],
         w["gn3gamma"], w["gn3beta"], w["gmask"], w["posb"]], 1)  # [200, 15]
    w["PRM_A"] = np.ascontiguousarray(prm[0:128]).astype(np.float32)
    w["PRM_B"] = np.ascontiguousarray(prm[128:200]).astype(np.float32)

    # posconv Toeplitz blocks: per chunk 14 mats ([114,128] fp16 block-diag):
    # 0..6 = Hh per dx, 7..13 = Hl per dx; lhsT[(d,h'),(d,h)] = W[dm0+d, h'-h+9, dx]
    posw = np.asarray(inp["pos_w"]).reshape(200, 19, 7).astype(np.float64)
    pwh = posw.astype(np.float16).astype(np.float64)
    pwl = posw - pwh
    hp_, h_ = np.meshgrid(np.arange(19), np.arange(19), indexing="ij")
    dy_ = hp_ - h_ + 9
    valid = (dy_ >= 0) & (dy_ < 19)
    dyc = np.clip(dy_, 0, 18)
    hst = np.zeros((NCHUNK * 14, 114, 128), np.float16)
    ci = 0
    for base, chunks in [(0, CHUNKS_A), (128, CHUNKS_B)]:
        for (off, ndm) in chunks:
            dm0 = base + off
            for hi_lo, W in [(0, pwh), (7, pwl)]:
                for dx in range(7):
                    M = np.zeros((114, 128), np.float64)
                    for d in range(ndm):
                        blk = np.where(valid, W[dm0 + d][dyc, dx], 0.0)
                        M[d * 19:(d + 1) * 19, d * 19:(d + 1) * 19] = blk
                    hst[ci * 14 + hi_lo + dx] = M.astype(np.float16)
            ci += 1
    w["Hst"] = hst.reshape(NCHUNK * 14 * 114, 128)

    # CB2T + nvec (fp64 host fold), fp16 hi/lo splits
    iw = np.asarray(inp["inp_w"]).astype(np.float64)
    cb = np.asarray(inp["codebook"]).astype(np.float64)
    cb2 = iw.T @ cb.T                                     # [200, 4096]
    nvec = cb @ np.asarray(inp["inp_b"]).astype(np.float64) - 0.5 * (cb * cb).sum(-1)
    cb2h, cb2l = _f16_split(cb2)
    nvh, nvl = _f16_split(nvec)
    w["cbhA"] = np.ascontiguousarray(cb2h[0:128])
    w["cblA"] = np.ascontiguousarray(cb2l[0:128])
    cbhB = np.zeros((97, KC), np.float16)
    cblB = np.zeros((97, KC), np.float16)
    cbhB[0:72] = cb2h[128:200]
    cbhB[96] = nvh
    cblB[0:72] = cb2l[128:200]
    cblB[96] = nvl
    w["cbhB"] = cbhB
    w["cblB"] = cblB

    # W2f rows (gathered by code index), outp_b folded in
    ow = np.asarray(inp["outp_w"]).astype(np.float64)
    w["W2f"] = (cb @ ow.T + np.asarray(inp["outp_b"]).astype(np.float64)).astype(np.float32)

    w["iota512"] = np.tile(np.arange(512, dtype=np.uint16), (128, 1))
    return w


def _build_nc(debug=False):
    import concourse.bass as bass
    import concourse.mybir as mybir
    import concourse.tile as tile
    from concourse import bacc

    f32 = mybir.dt.float32
    f32r = mybir.dt.float32r
    f16 = mybir.dt.float16
    u16 = mybir.dt.uint16
    u32 = mybir.dt.uint32
    Alu = mybir.AluOpType
    AF = mybir.ActivationFunctionType
    AX = mybir.AxisListType.X

    nc = bacc.Bacc("TRN2", target_bir_lowering=False, debug=False, num_devices=NCORES)

    di = {}
    di["xT"] = nc.dram_tensor("xT", [200, TOK], f32, kind="ExternalInput")
    di["FEW_A"] = nc.dram_tensor("FEW_A", [128, 802], f32, kind="ExternalInput")
    di["FEW_B"] = nc.dram_tensor("FEW_B", [72, 802], f32, kind="ExternalInput")
    di["PRM_A"] = nc.dram_tensor("PRM_A", [128, 15], f32, kind="ExternalInput")
    di["PRM_B"] = nc.dram_tensor("PRM_B", [72, 15], f32, kind="ExternalInput")
    di["spec_wT"] = nc.dram_tensor("spec_wT", [102, 200], f32, kind="ExternalInput")
    di["gmaskT"] = nc.dram_tensor("gmaskT", [5, 200], f32, kind="ExternalInput")
    di["Hst"] = nc.dram_tensor("Hst", [NCHUNK * 14 * 114, 128], f16, kind="ExternalInput")
    di["cbhA"] = nc.dram_tensor("cbhA", [128, KC], f16, kind="ExternalInput")
    di["cblA"] = nc.dram_tensor("cblA", [128, KC], f16, kind="ExternalInput")
    di["cbhB"] = nc.dram_tensor("cbhB", [97, KC], f16, kind="ExternalInput")
    di["cblB"] = nc.dram_tensor("cblB", [97, KC], f16, kind="ExternalInput")
    di["W2f"] = nc.dram_tensor("W2f", [KC, 200], f32, kind="ExternalInput")
    di["iota512"] = nc.dram_tensor("iota512", [128, 512], u16, kind="ExternalInput")
    p16d = nc.dram_tensor("p16d", [DM * 19, 2 * NB * 30], f16, kind="Internal")
    yd = nc.dram_tensor("yd", [DM * 19, NB * 30], f32, kind="Internal")
    # layouts: p16d rows (dm*19+h), cols (hl, w, b); yd rows same, cols (w, b)

    out_d = nc.dram_tensor("out", [TOK, 200], f32, kind="ExternalOutput")
    idx_d = nc.dram_tensor("idx", [128, 18], u32, kind="ExternalOutput")
    dbg = {}
    if debug:
        for nm in ["d_pe1", "d_pe2", "d_g1"]:
            dbg[nm] = nc.dram_tensor(nm, [200, TOK], f32, kind="ExternalOutput")
        dbg["d_sc0"] = nc.dram_tensor("d_sc0", [128, KC], f32, kind="ExternalOutput")
        dbg["d_am0"] = nc.dram_tensor("d_am0", [128, 40], f32, kind="ExternalOutput")

    TT = _tok_tiles()
    NS = _n_slices()

    with tile.TileContext(nc) as tc:
        with (
            tc.tile_pool(name="persist", bufs=1)) as persist, (
            tc.tile_pool(name="cbpool", bufs=1)) as cbpool, (
            tc.tile_pool(name="pepool", bufs=1)) as pepool:
            gidxu = persist.tile([128, 18], u32, name="gidxu")

            # score tables (loaded via the idle gpsimd queue; needed late)
            cbhA = cbpool.tile([128, KC], f16, name="cbhA")
            cblA = cbpool.tile([128, KC], f16, name="cblA")
            cbhB = cbpool.tile([97, KC], f16, name="cbhB")
            cblB = cbpool.tile([97, KC], f16, name="cblB")
            for nm, t in [("cbhA", cbhA), ("cblA", cblA), ("cbhB", cbhB), ("cblB", cblB)]:
                nc.gpsimd.dma_start(t[:], di[nm][:])

            # pe'' fp16 splits (score matmul lhs)
            pehA = pepool.tile([128, TOK], f16, name="pehA")
            pelA = pepool.tile([128, TOK], f16, name="pelA")
            pehB = pepool.tile([97, TOK], f16, name="pehB")
            pelB = pepool.tile([97, TOK], f16, name="pelB")
            nc.vector.memset(pehB[64:96, :], 0.0)
            nc.vector.memset(pehB[96:97, :], 1.0)
            nc.vector.memset(pelB[64:96, :], 0.0)
            nc.vector.memset(pelB[96:97, :], 0.0)

            # ---------------- Front end ----------------
            with (
                tc.tile_pool(name="fe2", bufs=1) as fe2,
                tc.tile_pool(name="fetmp", bufs=2) as fetmp,
                tc.tile_pool(name="fe1", bufs=1) as fe1,
                tc.tile_pool(name="feps", bufs=3, space="PSUM") as feps,
                tc.tile_pool(name="stps", bufs=1, space="PSUM") as stps,
            ):
                xA = fe1.tile([128, TOK], f32, name="xA", tag="xA")
                xB = fe1.tile([72, TOK], f32, name="xB", tag="xB")
                fewA = fe2.tile([128, 802], f32, name="fewA", tag="fewA")
                fewB = fe2.tile([72, 802], f32, name="fewB", tag="fewB")
                prmA = fetmp.tile([128, 15], f32, name="prmA", tag="prmA")
                prmB = fetmp.tile([72, 15], f32, name="prmB", tag="prmB")
                gmT = fetmp.tile([5, 200], f32, name="gmT", tag="gmT")
                nc.sync.dma_start(xA[:, 0:NSW], di["xT"][0:128, 0:NSW])
                nc.sync.dma_start(xB[:, 0:NSW], di["xT"][128:200, 0:NSW])
                nc.sync.dma_start(fewA[:], di["FEW_A"][:])
                nc.sync.dma_start(fewB[:], di["FEW_B"][:])
                for (n0, nsz) in NS[1:]:
                    nc.sync.dma_start(xA[:, n0:n0 + nsz], di["xT"][0:128, n0:n0 + nsz])
                    nc.sync.dma_start(xB[:, n0:n0 + nsz], di["xT"][128:200, n0:n0 + nsz])
                nc.scalar.dma_start(prmA[:], di["PRM_A"][:])
                nc.scalar.dma_start(prmB[:], di["PRM_B"][:])
                nc.scalar.dma_start(gmT[:], di["gmaskT"][:])
                gmA = prmA[:, 9:14]
                gmB = prmB[:, 9:14]

                g1A = fe2.tile([128, TOK], f32, name="g1A", tag="gA1")
                g1B = fe2.tile([72, TOK], f32, name="g1B", tag="gB1")
                g2A = fe2.tile([128, TOK], f32, name="g2A", tag="gA2")
                g2B = fe2.tile([72, TOK], f32, name="g2B", tag="gB2")
                g3A = fe2.tile([128, TOK], f32, name="g3A", tag="gA1")
                g3B = fe2.tile([72, TOK], f32, name="g3B", tag="gB1")

                def conv_gn_gelu(rhsA, rhsB, wcol, gi, outA, outB, dbg_g=None):
                    """rhs [128/72, TOK] f32 -> out = gelu(GN(conv)) f32."""
                    WA = fewA[:, wcol:wcol + 200]
                    WB = fewB[:, wcol:wcol + 200]
                    bcA = prmA[:, gi - 1:gi]
                    bcB = prmB[:, gi - 1:gi]
                    gamA = prmA[:, 1 + 2 * gi:2 + 2 * gi]
                    gamB = prmB[:, 1 + 2 * gi:2 + 2 * gi]
                    betA = prmA[:, 2 + 2 * gi:3 + 2 * gi]
                    betB = prmB[:, 2 + 2 * gi:3 + 2 * gi]

                    convA = fe1.tile([128, TOK], f32, name=f"convA{gi}", tag="convA")
                    convB = fe1.tile([72, TOK], f32, name=f"convB{gi}", tag="convB")
                    for (m0, msz, cdst, bc) in [(0, 128, convA, bcA),
                                                (128, 72, convB, bcB)]:
                        for (n0, nsz) in NS:
                            cps = feps.tile([128, NSW], f32, name="cps", tag="cps")
                            nc.tensor.matmul(cps[:msz, :nsz], WA[:, m0:m0 + msz],
                                             rhsA[:, n0:n0 + nsz], start=True, stop=False)
                            nc.tensor.matmul(cps[:msz, :nsz], WB[:, m0:m0 + msz],
                                             rhsB[:, n0:n0 + nsz], start=False, stop=True)
                            nc.scalar.activation(cdst[:, n0:n0 + nsz], cps[:msz, :nsz],
                                                 AF.Identity, bias=bc[:msz, :])

                    # GN stats: sum via DVE reduce, sumsq via Scalar Square-accum
                    stA = fetmp.tile([128, 8], f32, name=f"stA{gi}", tag="stA")
                    stB = fetmp.tile([72, 8], f32, name=f"stB{gi}", tag="stB")
                    scrA = fe1.tile([128, T1], f32, name=f"scrA{gi}", tag="scrA")
                    scrB = fe1.tile([72, T1], f32, name=f"scrB{gi}", tag="scrB")
                    for b in range(NB):
                        sl = slice(b * T1, (b + 1) * T1)
                        nc.vector.reduce_sum(stA[:, 2 * b:2 * b + 1], convA[:, sl], axis=AX)
                        nc.vector.reduce_sum(stB[:, 2 * b:2 * b + 1], convB[:, sl], axis=AX)
                        nc.scalar.activation(scrA[:], convA[:, sl], AF.Square,
                                             accum_out=stA[:, 2 * b + 1:2 * b + 2])
                        nc.scalar.activation(scrB[:], convB[:, sl], AF.Square,
                                             accum_out=stB[:, 2 * b + 1:2 * b + 2])
                    sps = stps.tile([5, 8], f32, name="sps", tag="stp")
                    nc.tensor.matmul(sps[:], gmA[:], stA[:], start=True, stop=False)
                    nc.tensor.matmul(sps[:], gmB[:], stB[:], start=False, stop=True)

                    st = fetmp.tile([5, 16], f32, name=f"st{gi}", tag="st")
                    st2 = fetmp.tile([5, 8], f32, name=f"st2{gi}", tag="st2")
                    NINV = 1.0 / (40 * T1)
                    nc.vector.tensor_scalar(st[:, 0:8], sps[:], NINV, None, op0=Alu.mult)
                    for b in range(NB):
                        nc.vector.tensor_copy(st2[:, b:b + 1], st[:, 2 * b:2 * b + 1])
                        nc.vector.tensor_mul(st[:, 8 + b:9 + b], st[:, 2 * b:2 * b + 1],
                                             st[:, 2 * b:2 * b + 1])
                        nc.vector.tensor_sub(st2[:, 4 + b:5 + b], st[:, 2 * b + 1:2 * b + 2],
                                             st[:, 8 + b:9 + b])
                    nc.vector.tensor_scalar(st2[:, 4:8], st2[:, 4:8], EPS, None, op0=Alu.add)
                    sqr = fetmp.tile([5, 4], f32, name=f"sqr{gi}", tag="sqr")
                    nc.scalar.activation(sqr[:], st2[:, 4:8], AF.Sqrt)
                    r0 = fetmp.tile([5, 4], f32, name=f"r0{gi}", tag="r0")
                    nc.vector.reciprocal(r0[:], sqr[:])
                    tn = fetmp.tile([5, 4], f32, name=f"tn{gi}", tag="tn")
                    nc.vector.tensor_mul(tn[:], r0[:], r0[:])
                    nc.vector.tensor_mul(tn[:], tn[:], st2[:, 4:8])
                    nc.vector.tensor_scalar(tn[:], tn[:], -0.5, 1.5, op0=Alu.mult, op1=Alu.add)
                    nc.vector.tensor_mul(st2[:, 4:8], r0[:], tn[:])

                    bpsA = stps.tile([128, 8], f32, name="bpsA", tag="stp")
                    bpsB = stps.tile([72, 8], f32, name="bpsB", tag="stp")
                    nc.tensor.matmul(bpsA[:], gmT[:, 0:128], st2[:], start=True, stop=True)
                    nc.tensor.matmul(bpsB[:], gmT[:, 128:200], st2[:], start=True, stop=True)
                    rgA = fetmp.tile([128, 8], f32, name=f"rgA{gi}", tag="rgA")
                    rgB = fetmp.tile([72, 8], f32, name=f"rgB{gi}", tag="rgB")
                    for (bps, rg, gmv, btv, prt) in [(bpsA, rgA, gamA, betA, 128),
                                                     (bpsB, rgB, gamB, betB, 72)]:
                        # rg[0:4] = rstd*gamma; rg[4:8] = beta - mean*rstd*gamma
                        nc.vector.tensor_scalar(rg[:prt, 0:4], bps[:prt, 4:8],
                                                gmv[:prt, :], None, op0=Alu.mult)
                        nc.vector.tensor_mul(rg[:prt, 4:8], bps[:prt, 0:4], rg[:prt, 0:4])
                        nc.vector.tensor_scalar(rg[:prt, 4:8], rg[:prt, 4:8],
                                                btv[:prt, :], None, op0=Alu.subtract)
                        nc.vector.tensor_scalar(rg[:prt, 4:8], rg[:prt, 4:8], -1.0, None,
                                                op0=Alu.mult)
                    # fused GN-apply + exact GELU on Scalar engine
                    for b in range(NB):
                        sl = slice(b * T1, (b + 1) * T1)
                        nc.scalar.activation(outA[:, sl], convA[:, sl], AF.Gelu,
                                             scale=rgA[:, b:b + 1], bias=rgA[:, 4 + b:5 + b])
                        nc.scalar.activation(outB[:, sl], convB[:, sl], AF.Gelu,
                                             scale=rgB[:, b:b + 1], bias=rgB[:, 4 + b:5 + b])
                    if dbg_g is not None:
                        nc.sync.dma_start(dbg_g[0:128, :], outA[:])
                        nc.sync.dma_start(dbg_g[128:200, :], outB[:])

                FA = fewA[:, 600:802]
                FB = fewB[:, 600:802]
                reT = fe2.tile([101, TOK], f32, name="reT", tag="gA2")
                imT = fe2.tile([101, TOK], f32, name="imT", tag="gB2x",
                               padded_shape=[128, TOK])

                conv_gn_gelu(xA, xB, 0, 1, g1A, g1B, dbg.get("d_g1"))
                conv_gn_gelu(g1A, g1B, 200, 2, g2A, g2B)
                conv_gn_gelu(g2A, g2B, 400, 3, g3A, g3B)
                for (m0, dst) in [(0, reT), (101, imT)]:
                    for (n0, nsz) in NS:
                        cps = feps.tile([128, NSW], f32, name="cpsf", tag="cps")
                        nc.tensor.matmul(cps[:101, :nsz], FA[:, m0:m0 + 101],
                                         xA[:, n0:n0 + nsz], start=True, stop=False)
                        nc.tensor.matmul(cps[:101, :nsz], FB[:, m0:m0 + 101],
                                         xB[:, n0:n0 + nsz], start=False, stop=True)
                        nc.scalar.activation(dst[:, n0:n0 + nsz], cps[:101, :nsz], AF.Copy)
                nc.vector.tensor_mul(reT[:], reT[:], reT[:])
                nc.vector.tensor_mul(imT[:], imT[:], imT[:])
                nc.vector.tensor_add(reT[:], reT[:], imT[:])

                specA = fe1.tile([102, TOK], f32, name="specA", tag="convA")
                nc.vector.memset(specA[96:102, :], 1.0)
                epsb = fetmp.tile([101, 1], f32, name="epsb", tag="gam")
                nc.vector.memset(epsb[:], 1e-30)
                nc.scalar.activation(specA[0:101, :], reT[:], AF.Sqrt, bias=epsb[:])
                swT = fetmp.tile([102, 200], f32, name="swT", tag="WB")
                nc.scalar.dma_start(swT[:], di["spec_wT"][0:102, :])
                pe1A = fe2.tile([128, TOK], f32, name="pe1A", tag="gA2p",
                                padded_shape=[128, TOK])
                pe1B = fe2.tile([72, TOK], f32, name="pe1B", tag="gB2p",
                                padded_shape=[128, TOK])
                for (m0, msz, gsrc, pdst) in [(0, 128, g3A, pe1A), (128, 72, g3B, pe1B)]:
                    for (n0, nsz) in NS:
                        cps = feps.tile([128, NSW], f32, name="cpss", tag="cps")
                        nc.tensor.matmul(cps[:msz, :nsz], swT[:, m0:m0 + msz],
                                         specA[:, n0:n0 + nsz], start=True, stop=True)
                        nc.vector.scalar_tensor_tensor(
                            pdst[:, n0:n0 + nsz], cps[:msz, :nsz], 1.0,
                            gsrc[:msz, n0:n0 + nsz],
                            op0=Alu.mult, op1=Alu.add)
                if debug:
                    nc.sync.dma_start(dbg["d_pe1"][0:128, :], pe1A[:])
                    nc.sync.dma_start(dbg["d_pe1"][128:200, :], pe1B[:])

                # ---------------- pos conv (Toeplitz h-matmuls) ----------------
                pbA = prmA[:, 14:15]
                pbB = prmB[:, 14:15]
                # fp16 hi/lo of pe1 stored [dm, h, w, b] (batch innermost) so the
                # DRAM staging runs are (w, b) = 240B contiguous
                pe16A = fe1.tile([128, 19, 30, NB], f16, name="pe16A", tag="scrA2",
                                 padded_shape=[128, 19, 30, NB])
                pe16B = fe1.tile([72, 19, 30, NB], f16, name="pe16B", tag="scrB2",
                                 padded_shape=[128, 19, 30, NB])
                pl16A = fe1.tile([128, 19, 30, NB], f16, name="pl16A", tag="scrA3",
                                 padded_shape=[128, 19, 30, NB])
                pl16B = fe1.tile([72, 19, 30, NB], f16, name="pl16B", tag="scrB3",
                                 padded_shape=[128, 19, 30, NB])
                for (p16, pl16, pe1x, nb) in [(pe16A, pl16A, pe1A, 128),
                                              (pe16B, pl16B, pe1B, 72)]:
                    hv16 = p16[:].rearrange("d h w b -> d b (h w)")
                    lv16 = pl16[:].rearrange("d h w b -> d b (h w)")
                    pv = pe1x[:].rearrange("d (b hw) -> d b hw", b=NB)
                    nc.scalar.activation(hv16, pv, AF.Copy)
                    nc.vector.tensor_tensor(lv16, pv, hv16, op=Alu.subtract)
                posPA = fe2.tile([128, 19, 30, NB], f32, name="posPA", tag="gA1")
                posPB = fe2.tile([72, 19, 30, NB], f32, name="posPB", tag="gB1")

                # stage Xh/Xl to DRAM: [(dm h), (hl, w, b)]; 4 DMAs, 240B runs
                p16v = p16d[:].rearrange("(d h) (l n) -> d h l n", h=19, l=2)
                ydv = yd[:].rearrange("(d h) n -> d h n", h=19)
                for (hl, srcA, srcB) in [(0, pe16A, pe16B), (1, pl16A, pl16B)]:
                    nc.sync.dma_start(
                        p16v[0:128, :, hl, :],
                        srcA[:].rearrange("d h w b -> d h (w b)"))
                    nc.sync.dma_start(
                        p16v[128:200, :, hl, :],
                        srcB[:].rearrange("d h w b -> d h (w b)"))
                with (
                    tc.tile_pool(name="pcx", bufs=7) as pcx,
                    tc.tile_pool(name="pch", bufs=6) as pch,
                    tc.tile_pool(name="pcy", bufs=4) as pcy,
                    tc.tile_pool(name="pcps", bufs=4, space="PSUM") as pcps,
                ):
                    hview = di["Hst"][:].rearrange("(c p) m -> c p m", p=114)
                    p16r = p16d[:].rearrange("r (l n) -> r l n", l=2)
                    ci = 0
                    for (base, chunks) in [(0, CHUNKS_A), (128, CHUNKS_B)]:
                        for (off, ndm) in chunks:
                            rows = ndm * 19
                            r0 = (base + off) * 19
                            # Xc layout [114, hl, w(36 padded), b]
                            Xc = pcx.tile([114, 2, 36, NB], f16, name="Xc", tag="Xc")
                            nc.vector.memset(Xc[:, :, 0:3, :], 0.0)
                            nc.vector.memset(Xc[:, :, 33:36, :], 0.0)
                            if rows < 114:
                                nc.vector.memset(Xc[32:64, :, :, :], 0.0)
                                nc.vector.memset(Xc[64:96, :, :, :], 0.0)
                                nc.vector.memset(Xc[96:114, :, :, :], 0.0)
                            nc.sync.dma_start(
                                Xc[0:rows, :, 3:33, :].rearrange("p l w b -> p l (w b)"),
                                p16r[r0:r0 + rows, :, :])
                            Hc = pch.tile([114, 14, 128], f16, name="Hc", tag="Hc")
                            nc.gpsimd.dma_start(
                                Hc[:], hview[14 * ci:14 * ci + 14, :, :]
                                .rearrange("c p m -> p c m"))
                            pc = pcps.tile([128, 30, NB], f32, name="pc", tag="pc")
                            # Hh*Xh + Hh*Xl (same weights back-to-back), + Hl*Xh
                            for dx in range(7):
                                nc.tensor.matmul(pc[:], Hc[:, dx, :],
                                                 Xc[:, 0, dx:dx + 30, :],
                                                 start=(dx == 0), stop=False)
                                nc.tensor.matmul(pc[:], Hc[:, dx, :],
                                                 Xc[:, 1, dx:dx + 30, :],
                                                 start=False, stop=False)
                            for dx in range(7):
                                nc.tensor.matmul(pc[:], Hc[:, 7 + dx, :],
                                                 Xc[:, 0, dx:dx + 30, :],
                                                 start=False, stop=(dx == 6))
                            Yc = pcy.tile([114, 30, NB], f32, name="Yc", tag="Yc")
                            nc.scalar.activation(Yc[:], pc[0:114, :, :], AF.Copy)
                            nc.scalar.dma_start(
                                yd[r0:r0 + rows, :],
                                Yc[:rows].rearrange("p w b -> p (w b)"))
                            ci += 1
                    nc.sync.dma_start(
                        posPA[:].rearrange("d h w b -> d h (w b)"), ydv[0:128, :, :])
                    nc.sync.dma_start(
                        posPB[:].rearrange("d h w b -> d h (w b)"), ydv[128:200, :, :])

                # pe'' = pe1 + pos + posb; then fp16 hi/lo split
                pe2A = fe2.tile([128, TOK], f32, name="pe2A", tag="gA2")
                pe2B = fe2.tile([72, TOK], f32, name="pe2B", tag="gB2")
                nc.vector.scalar_tensor_tensor(
                    pe2A[:].rearrange("d (b hw) -> d b hw", b=NB),
                    posPA[:].rearrange("d h w b -> d b (h w)"), pbA[:, 0:1],
                    pe1A[:].rearrange("d (b hw) -> d b hw", b=NB),
                    op0=Alu.add, op1=Alu.add)
                nc.vector.scalar_tensor_tensor(
                    pe2B[:].rearrange("d (b hw) -> d b hw", b=NB),
                    posPB[:].rearrange("d h w b -> d b (h w)"), pbB[:, 0:1],
                    pe1B[:].rearrange("d (b hw) -> d b hw", b=NB),
                    op0=Alu.add, op1=Alu.add)
                if debug:
                    nc.sync.dma_start(dbg["d_pe2"][0:128, :], pe2A[:])
                    nc.sync.dma_start(dbg["d_pe2"][128:200, :], pe2B[:])
                nc.scalar.activation(pehA[:], pe2A[:], AF.Copy)
                nc.vector.tensor_sub(pelA[:], pe2A[:], pehA[:])
                nc.scalar.activation(pehB[0:72, :], pe2B[:], AF.Copy)
                nc.vector.tensor_sub(pelB[0:72, :], pe2B[:], pehB[0:72, :])

            # ------- scores: 3-term fp16, 2-stage argmax, W2f gather
            with (
                tc.tile_pool(name="sce", bufs=2) as sce,
                tc.tile_pool(name="gat", bufs=3) as gat,
                tc.tile_pool(name="scps", bufs=8, space="PSUM") as scps,
            ):
                for ti, (t0, tsz) in enumerate(TT):
                    tsl = slice(t0, t0 + tsz)
                    sc = sce.tile([128, KC], f32, name="sc", tag="sc")
                    for kc in range(8):
                        csl = slice(kc * 512, (kc + 1) * 512)
                        sps_ = scps.tile([128, 512], f32, name="sps_", tag="sps")
                        seq = [
                            (pehA, cbhA), (pelA, cbhA), (pehA, cblA),
                            (pehB, cbhB), (pelB, cbhB), (pehB, cblB),
                        ]
                        for i, (lh, rh) in enumerate(seq):
                            nc.tensor.matmul(sps_[:tsz, :], lh[:, tsl], rh[:, csl],
                                             start=(i == 0), stop=(i == len(seq) - 1))
                        nc.scalar.activation(sc[:tsz, csl], sps_[:tsz, :], AF.Copy)
                    # argmax: top-8 values + index find (hidden under PE)
                    m8 = gat.tile([128, 8], f32, name="m8", tag="m8")
                    mi8 = gat.tile([128, 8], u32, name="mi8", tag="mi8")
                    nc.vector.max(m8[:tsz, :], sc[:tsz, :])
                    nc.vector.max_index(mi8[:tsz, :], m8[:tsz, :], sc[:tsz, :])
                    nc.vector.tensor_copy(gidxu[:tsz, ti:ti + 1], mi8[:tsz, 0:1])
                    if debug and ti == 0:
                        nc.sync.dma_start(dbg["d_sc0"][:], sc[:])
                    go = gat.tile([128, 200], f32, name="go", tag="go")
                    nc.gpsimd.indirect_dma_start(
                        out=go[:tsz, :], out_offset=None,
                        in_=di["W2f"][:],
                        in_offset=bass.IndirectOffsetOnAxis(
                            ap=gidxu[:tsz, ti:ti + 1], axis=0))
                    nc.sync.dma_start(out_d[t0:t0 + tsz, :], go[:tsz, :])
                nc.sync.dma_start(idx_d[:], gidxu[:])

    nc.compile()
    return nc


def _prep_inputs(inp):
    w = build_host_weights(inp)
    x = np.asarray(inp["x"], np.float32).reshape(B * T1, 200)
    shared = {}
    for k in ["FEW_A", "FEW_B", "PRM_A", "PRM_B", "spec_wT", "gmaskT",
              "Hst", "cbhA", "cblA", "cbhB", "cblB", "W2f", "iota512"]:
        shared[k] = np.ascontiguousarray(w[k])
    in_maps = []
    for c in range(NCORES):
        m = dict(shared)
        m["xT"] = np.ascontiguousarray(x[c * TOK:(c + 1) * TOK].T)
        in_maps.append(m)
    return in_maps


def run(inp, debug=False, trace=False, **kw):
    global _COMPILED
    from concourse.bass_utils import run_bass_kernel_spmd
    if _COMPILED is None or _COMPILED[1] != debug:
        _COMPILED = (_build_nc(debug=debug), debug)
    nc = _COMPILED[0]
    in_maps = _prep_inputs(inp)
    res = run_bass_kernel_spmd(nc, in_maps, core_ids=list(range(NCORES)), trace=trace, **kw)
    return res


def kernel(**inputs):
    res = run(inputs)
    out = np.concatenate([r["out"] for r in res.results], 0)
    return out.reshape(B, CH, NP_, DM)
